# revision 33
# baseline (speedup 1.0000x reference)
"""CMAttention Trainium2 kernel (Bass/Tile), data-parallel over batch on 8 cores.

Reference computation (per batch b, per head h, d=64, n=1024):
  q = inp @ Wq.T + bq                    (split heads)
  k = [ctx @ Wk.T + bk ; sqrt(64)*mk]    ctx = [x;y], 2051 keys
  cross = softmax(q k^T / 8) @ Wf.T + bf          [n, 1027]
  Sk = [inp @ Wk.T + bk ; sqrt(64)*smk]  1027 self keys
  self = softmax(q Sk^T / 8)                       [n, 1027]
  Sv = [inp @ Wv.T + bv ; sqrt(3)*smv]             [1027, 64]
  out_h = (cross + self) @ Sv
  out = concat_h(out_h) @ Wo.T + bo

Kernel strategy (everything transposed: features/keys on partitions):
  - All activations/weights pre-transposed and cast to bf16 on host; bf16
    matmuls with fp32 PSUM accumulation throughout (norm rel err ~6e-3).
  - dots^T[j, i] computed directly via matmul(lhsT=KT_h, rhs=QT_h) (K=64),
    both 512-wide i-chunks into one [128, 1024] PSUM tile -> a single
    Exp per j-tile (ACT per-instruction overhead amortized; ACT is the
    co-bottleneck engine with PE).
  - Unnormalized softmax: E = exp(dots*scale) (bf16), no max subtraction
    (arguments are O(+-6), safely in fp32/bf16 exp range).
  - KEY TRICK: the reference's huge fuse matmul softmax(QK^T) @ Wf.T
    (276 of 345 total GFLOPs) is never materialized.  Since its only
    consumer is the final "@ Sv" contraction, associativity gives
      (E @ Wf.T) @ Sv = E @ (Wf.T @ Sv) = E @ G,   G = Wf.T @ Sv
    where G is [2051, 64] per head - computed once per head for ~16x
    less work.  G carries a ones column so U1's PSUM row 64 = Z (the
    softmax denominator); normalization moves past both matmuls and is
    applied as a per-row reciprocal broadcast at the [64, 1024] stage.
  - U1^T = G^T-contract-E (cross branch), U2^T = Sv^T-contract-E2 (self
    branch, ones column -> Z2).  contrib = U1/Z + U2/Z2 via fp32
    reciprocal + gpsimd partition_broadcast + DVE multiplies.
  - Head 0's dots+exp are emitted before the Sv/G phase so the ACT
    engine starts early and stays saturated.
  - Final projection contracts out^T with Wo^T; bias row (bo + bf@Sv@Wo^T)
    added via a K=1 ones matmul.  All biases (zero in this problem's
    setup_inputs, but implemented faithfully and tested nonzero) ride
    per-partition tensor_scalar adds or host-prepared broadcast tiles.
"""
import numpy as np
import ml_dtypes
from contextlib import ExitStack

import concourse.bass as bass
import concourse.tile as tile
from concourse import bacc, mybir
from concourse import bass_utils

F32 = mybir.dt.float32
BF16 = mybir.dt.bfloat16
bfnp = ml_dtypes.bfloat16
AF = mybir.ActivationFunctionType
ALU = mybir.AluOpType

B = 8
N = 1024
DIM = 512
HEADS = 8
DH = 64
M = 3
SCALE = 0.125
NK = 2 * N + M          # 2051 cross keys
NF = N + M              # 1027 fuse outputs / self keys
NKT = 17                # cross j-tiles: 16 full + [3 mem + 1 bias row]
NFT = 9                 # self j2-tiles: 8 full + 3 mem
CH = 2                  # i-chunks of 512
CW = 512


def build_kernel(tc):
    nc = tc.nc
    d = {}
    def din(name, shape, dt=BF16):
        d[name] = nc.dram_tensor(name, shape, dt, kind="ExternalInput").ap()
    din("inpT", [DIM, N])
    din("ctxT", [DIM, 2 * N])
    din("wqT", [DIM, DIM])
    din("wkT", [DIM, DIM])
    din("wvT", [DIM, DIM])
    din("woT", [DIM, DIM])
    din("wfN", [NF, NK])         # Wf natural [1027, 2051]
    din("memKT", [DIM, M])
    din("memSkT", [DIM, M])
    din("svmemB", [M, 520])      # rows [Sv1024..26], ones col
    din("b_sv", [128, DIM])      # bkv_v broadcast tile
    din("bq", [DIM, 1], F32)
    din("bk", [DIM, 1], F32)
    din("bfcol", [NF, 1])        # bf
    din("borow", [1, DIM])
    out_d = nc.dram_tensor("out", [N, DIM], F32, kind="ExternalOutput").ap()

    ctx = ExitStack()
    with ctx:
        pers = ctx.enter_context(tc.tile_pool(name="pers", bufs=1))
        ppd = ctx.enter_context(tc.tile_pool(name="ppd", bufs=2, space="PSUM"))
        ppu = ctx.enter_context(tc.tile_pool(name="ppu", bufs=1, space="PSUM"))

        # ---------------- persistent SBUF ----------------
        woT = [pers.tile([128, DIM], BF16, tag=f"woT{t}", name=f"woT{t}") for t in range(4)]
        Gst = [pers.tile([128 if t < 16 else M, 520], BF16, tag=f"Gst{t}", name=f"Gst{t}")
               for t in range(NKT)]
        QT = [pers.tile([128, N], BF16, tag=f"QT{t}", name=f"QT{t}") for t in range(4)]
        KT = [pers.tile([128, NK], BF16, tag=f"KT{t}", name=f"KT{t}") for t in range(4)]
        SkT = [pers.tile([128, NF], BF16, tag=f"SkT{t}", name=f"SkT{t}") for t in range(4)]
        Sv = [pers.tile([128, 520], BF16, tag=f"Sv{t}", name=f"Sv{t}") for t in range(8)]
        svB = pers.tile([M, 520], BF16, tag="svB", name="svB")
        outT = [pers.tile([128, N], BF16, tag=f"outT{t}", name=f"outT{t}") for t in range(4)]
        bsv = pers.tile([128, DIM], BF16, tag="bsv", name="bsv")
        bq = [pers.tile([128, 1], F32, tag=f"bq{t}", name=f"bq{t}") for t in range(4)]
        bk = [pers.tile([128, 1], F32, tag=f"bk{t}", name=f"bk{t}") for t in range(4)]
        bfc = [pers.tile([128 if t < 8 else M, 1], BF16, tag=f"bfc{t}", name=f"bfc{t}")
               for t in range(NFT)]
        borow = pers.tile([1, DIM], BF16, tag="borow", name="borow")
        ones128 = pers.tile([1, 128], BF16, tag="ones128", name="ones128")
        wT = [pers.tile([128, 1], BF16, tag=f"wT{t}", name=f"wT{t}") for t in range(4)]
        wob = pers.tile([1, DIM], BF16, tag="wob", name="wob")

        # ---------------- input DMAs (persistent) ----------------
        for t in range(4):
            nc.sync.dma_start(woT[t][:], d["woT"][128 * t:128 * t + 128, :])
            nc.sync.dma_start(bq[t][:], d["bq"][128 * t:128 * t + 128, :])
            nc.sync.dma_start(bk[t][:], d["bk"][128 * t:128 * t + 128, :])
            nc.sync.dma_start(KT[t][:, 2 * N:], d["memKT"][128 * t:128 * t + 128, :])
            nc.sync.dma_start(SkT[t][:, N:], d["memSkT"][128 * t:128 * t + 128, :])
        for t in range(NFT):
            r0 = 128 * t
            r1 = min(r0 + 128, NF)
            nc.sync.dma_start(bfc[t][:], d["bfcol"][r0:r1, :])
        nc.sync.dma_start(svB[:], d["svmemB"][:])
        nc.sync.dma_start(bsv[:], d["b_sv"][:])
        nc.sync.dma_start(borow[:], d["borow"][:])
        nc.vector.memset(ones128[:], 1.0)

        work = ctx.enter_context(tc.tile_pool(name="work", bufs=2))
        epool = ctx.enter_context(tc.tile_pool(name="epool", bufs=1))
        fpool = ctx.enter_context(tc.tile_pool(name="fpool", bufs=2))

        def dots_phase(h):
            """dots^T + exp for one head; both i-chunks in one [128,1024] psum."""
            ht, sd = divmod(h, 2)
            hp = 64 * sd
            E, E2 = {}, {}
            for src, nt, full, tagc, store in ((KT, NKT, 16, "E", E),
                                               (SkT, NFT, 8, "F", E2)):
                for t in range(nt):
                    mw = 128 if t < full else M
                    ps = ppd.tile([128, 2 * CW], F32, tag="pdE", name="pdE")
                    for c in range(CH):
                        nc.tensor.matmul(ps[0:mw, CW * c:CW * c + CW],
                                         src[ht][hp:hp + 64, 128 * t:128 * t + mw],
                                         QT[ht][hp:hp + 64, CW * c:CW * c + CW],
                                         start=True, stop=True)
                    pool = fpool if (tagc == "F" and t < 8) else epool
                    e = pool.tile([128 if t < full else M, 2 * CW], BF16,
                                  tag=f"{tagc}{t}", name=f"{tagc}{t}")
                    store[t] = e
                    nc.scalar.activation(e[0:mw, :], ps[0:mw, :], AF.Exp,
                                         scale=SCALE)
            return E, E2

        def u_phase(h, E, E2):
            ht, sd = divmod(h, 2)
            hp = 64 * sd
            # U1^T = G^T-contract-E; ones col -> row 64 = Z
            pu1 = ppu.tile([65, 2 * CW], F32, tag="pu1", name="pu1")
            for t in range(NKT):
                kw = 128 if t < 16 else M
                for c in range(CH):
                    cs = slice(CW * c, CW * c + CW)
                    nc.tensor.matmul(pu1[:, cs],
                                     Gst[t][0:kw, 65 * h:65 * h + 65],
                                     E[t][0:kw, cs],
                                     start=(t == 0), stop=(t == NKT - 1))
            rzb = work.tile([64, 2 * CW], F32, tag="rzb", name="rzb")
            nc.vector.reciprocal(rzb[0:1, :], pu1[64:65, :])
            nc.gpsimd.partition_broadcast(rzb[:], rzb[0:1, :])
            # U2^T with ones col -> row 64 = Z2
            pu2 = ppu.tile([65, 2 * CW], F32, tag="pu2", name="pu2")
            for t in range(NFT):
                kw = 128 if t < 8 else M
                lhs = (Sv[t] if t < 8 else svB)[0:kw, 65 * h:65 * h + 65]
                for c in range(CH):
                    cs = slice(CW * c, CW * c + CW)
                    nc.tensor.matmul(pu2[:, cs], lhs, E2[t][0:kw, cs],
                                     start=(t == 0), stop=(t == NFT - 1))
            rz2b = work.tile([64, 2 * CW], F32, tag="rz2b", name="rz2b")
            nc.vector.reciprocal(rz2b[0:1, :], pu2[64:65, :])
            nc.gpsimd.partition_broadcast(rz2b[:], rz2b[0:1, :])
            tmp = work.tile([64, 2 * CW], BF16, tag="tmp", name="tmp")
            nc.vector.tensor_tensor(tmp[:], pu2[0:64, :], rz2b[:], ALU.mult)
            tmp1 = work.tile([64, 2 * CW], BF16, tag="tmp1", name="tmp1")
            nc.vector.tensor_tensor(tmp1[:], pu1[0:64, :], rzb[:], ALU.mult)
            nc.vector.tensor_tensor(outT[ht][hp:hp + 64, :], tmp1[:],
                                    tmp[:], ALU.add)

        # ---------------- projections (scoped pool, released after) --------
        with tc.tile_pool(name="projp", bufs=1) as projp:
            inpT = [projp.tile([128, N], BF16, tag=f"inpT{t}", name=f"inpT{t}")
                    for t in range(4)]
            ctxT = [projp.tile([128, 2 * N], BF16, tag=f"ctxT{t}", name=f"ctxT{t}")
                    for t in range(4)]
            wqT = [projp.tile([128, DIM], BF16, tag=f"wqT{t}", name=f"wqT{t}")
                   for t in range(4)]
            wkT = [projp.tile([128, DIM], BF16, tag=f"wkT{t}", name=f"wkT{t}")
                   for t in range(4)]
            wvT = [projp.tile([128, DIM], BF16, tag=f"wvT{t}", name=f"wvT{t}")
                   for t in range(4)]
            for t in range(4):
                nc.sync.dma_start(inpT[t][:], d["inpT"][128 * t:128 * t + 128, :])
                nc.sync.dma_start(wqT[t][:], d["wqT"][128 * t:128 * t + 128, :])
                nc.sync.dma_start(wkT[t][:], d["wkT"][128 * t:128 * t + 128, :])
                nc.sync.dma_start(ctxT[t][:], d["ctxT"][128 * t:128 * t + 128, :])
                nc.sync.dma_start(wvT[t][:], d["wvT"][128 * t:128 * t + 128, :])

            def proj_q(t):
                for c in range(CH):     # i chunk
                    ps = ppd.tile([128, 2 * CW], F32, tag="pdE", name="pdE")[:, 0:CW]
                    for k in range(4):
                        nc.tensor.matmul(ps[:], wqT[k][:, 128 * t:128 * t + 128],
                                         inpT[k][:, CW * c:CW * c + CW],
                                         start=(k == 0), stop=(k == 3))
                    nc.vector.tensor_scalar(QT[t][:, CW * c:CW * c + CW], ps[:],
                                            bq[t][:], None, ALU.add)

            def proj_k(t):
                for c in range(4):      # 2N = 4 chunks
                    ps = ppd.tile([128, 2 * CW], F32, tag="pdE", name="pdE")[:, 0:CW]
                    for k in range(4):
                        nc.tensor.matmul(ps[:], wkT[k][:, 128 * t:128 * t + 128],
                                         ctxT[k][:, CW * c:CW * c + CW],
                                         start=(k == 0), stop=(k == 3))
                    nc.vector.tensor_scalar(KT[t][:, CW * c:CW * c + CW], ps[:],
                                            bk[t][:], None, ALU.add)

            def proj_sk(t):
                for c in range(CH):
                    ps = ppd.tile([128, 2 * CW], F32, tag="pdE", name="pdE")[:, 0:CW]
                    for k in range(4):
                        nc.tensor.matmul(ps[:], wkT[k][:, 128 * t:128 * t + 128],
                                         inpT[k][:, CW * c:CW * c + CW],
                                         start=(k == 0), stop=(k == 3))
                    nc.vector.tensor_scalar(SkT[t][:, CW * c:CW * c + CW], ps[:],
                                            bk[t][:], None, ALU.add)

            # tile 0 of Q/K/Sk first so head-0 dots+exp can start after ~32
            # matmuls; the hoisted dots keep ACT busy through the rest of
            # the projections and the Sv/G phases.
            proj_q(0); proj_k(0); proj_sk(0)
            E_h0, E2_h0 = dots_phase(0)
            for t in range(1, 4):
                proj_q(t)
            for t in range(1, 4):
                proj_k(t)
            for t in range(1, 4):
                proj_sk(t)

            # Sv natural [i, dv]: lhsT = inpT (c -> i), rhs = wvT (c -> dv)
            for t in range(8):          # i tile
                ps = ppd.tile([128, 2 * CW], F32, tag="pdE", name="pdE")[:, 0:CW]
                for k in range(4):
                    nc.tensor.matmul(ps[:], inpT[k][:, 128 * t:128 * t + 128],
                                     wvT[k][:], start=(k == 0), stop=(k == 3))
                # value cols (strided by 65) = psum + bias_bcast; ones cols = 1
                vcols = Sv[t][:].rearrange("p (h c) -> p h c", h=8)[:, :, 0:64]
                nc.vector.tensor_tensor(vcols, ps[:], bsv[:], ALU.add)
                ocols = Sv[t][:].rearrange("p (h c) -> p h c", h=8)[:, :, 64:65]
                nc.vector.memset(ocols, 1.0)

        with tc.tile_pool(name="projq", bufs=1) as projq:
            wfN = [projq.tile([128 if t < 8 else M, NK], BF16,
                              tag=f"wfN{t}", name=f"wfN{t}") for t in range(NFT)]
            for t in range(NFT):
                r0 = 128 * t
                r1 = min(r0 + 128, NF)
                nc.sync.dma_start(wfN[t][:], d["wfN"][r0:r1, :])
            # G = Wf.T @ Sv (all heads at once): G[j, (h,d)] strided like Sv.
            for t in range(NKT):        # j tile
                mw = 128 if t < 16 else M
                ps = ppd.tile([128, 2 * CW], F32, tag="pdE", name="pdE")[:, 0:CW]
                for k in range(NFT):
                    kw = 128 if k < 8 else M
                    rhs = (Sv[k] if k < 8 else svB)[0:kw].rearrange(
                        "p (h c) -> p h c", h=8)[:, :, 0:64]
                    nc.tensor.matmul(ps[0:mw, :],
                                     wfN[k][0:kw, 128 * t:128 * t + mw],
                                     rhs, start=(k == 0), stop=(k == NFT - 1))
                gv = Gst[t][0:mw].rearrange("p (h c) -> p h c", h=8)[:, :, 0:64]
                nc.vector.tensor_copy(gv, ps[0:mw, :])
                go = Gst[t][0:mw].rearrange("p (h c) -> p h c", h=8)[:, :, 64:65]
                nc.vector.memset(go, 1.0)

        # ---------------- attention ----------------
        u_phase(0, E_h0, E2_h0)
        for h in range(1, HEADS):
            E_h, E2_h = dots_phase(h)
            u_phase(h, E_h, E2_h)

        # ---------------- bias terms: w = bf @ Sv per head ----------------
        for h in range(HEADS):
            ht, hp = divmod(h, 2)
            hp *= 64
            pw = ppu.tile([65, CW], F32, tag="pu1", name="pu1")[0:64]
            for t in range(NFT):
                kw = 128 if t < 8 else M
                lhs = (Sv[t] if t < 8 else svB)[0:kw, 65 * h:65 * h + 64]
                nc.tensor.matmul(pw[:, 0:1], lhs, bfc[t][0:kw, :],
                                 start=(t == 0), stop=(t == NFT - 1))
            nc.vector.tensor_copy(wT[ht][hp:hp + 64, :], pw[:, 0:1])
        prow = ppu.tile([65, CW], F32, tag="pu1", name="pu1")[0:64]
        for k in range(4):
            nc.tensor.matmul(prow[0:1, :], wT[k][:], woT[k][:],
                             start=(k == 0), stop=(k == 3))
        nc.vector.tensor_tensor(wob[:], prow[0:1, :], borow[:], ALU.add)

        # ---------------- final projection ----------------
        for t in range(8):
            ps = ppd.tile([128, 2 * CW], F32, tag="pdE", name="pdE")[:, 0:CW]
            for k in range(4):
                nc.tensor.matmul(ps[:], outT[k][:, 128 * t:128 * t + 128],
                                 woT[k][:], start=(k == 0), stop=False)
            nc.tensor.matmul(ps[:], ones128[:], wob[:], start=False, stop=True)
            o_sb = work.tile([128, CW], F32, tag="osb", name="osb")
            nc.vector.tensor_copy(o_sb[:], ps[:])
            nc.sync.dma_start(out_d[128 * t:128 * t + 128, :], o_sb[:])


# ---------------------------------------------------------------------------
# host side
# ---------------------------------------------------------------------------
_CACHE = {}


def _get_nc():
    if "nc" not in _CACHE:
        nc = bacc.Bacc("TRN2", target_bir_lowering=False, debug=False,
                       enable_asserts=False, num_devices=B)
        with tile.TileContext(nc) as tc:
            build_kernel(tc)
        nc.compile()
        _CACHE["nc"] = nc
    return _CACHE["nc"]


def _prep_shared(Wq, bq, Wkv, bkv, Wf, bf, Wo, bo, m_k, m_v, Sm_k, Sm_v):
    f = np.float32
    s = {}
    s["wqT"] = np.ascontiguousarray(np.asarray(Wq, f).T).astype(bfnp)
    s["wkT"] = np.ascontiguousarray(np.asarray(Wkv, f)[:DIM].T).astype(bfnp)
    s["wvT"] = np.ascontiguousarray(np.asarray(Wkv, f)[DIM:].T).astype(bfnp)
    s["woT"] = np.ascontiguousarray(np.asarray(Wo, f).T).astype(bfnp)

    s["wfN"] = np.ascontiguousarray(np.asarray(Wf, f)).astype(bfnp)
    bfv = np.asarray(bf, f)

    mkv = (np.sqrt(DH) * np.broadcast_to(np.asarray(m_k, f), (1, M, DIM))
           ).reshape(HEADS, M, DH)
    smk = (np.sqrt(DH) * np.broadcast_to(np.asarray(Sm_k, f), (1, M, DIM))
           ).reshape(HEADS, M, DH)
    smv = (np.sqrt(M) * np.broadcast_to(np.asarray(Sm_v, f), (1, M, DIM))
           ).reshape(HEADS, M, DH)
    s["memKT"] = np.ascontiguousarray(
        mkv.transpose(0, 2, 1).reshape(DIM, M)).astype(bfnp)
    s["memSkT"] = np.ascontiguousarray(
        smk.transpose(0, 2, 1).reshape(DIM, M)).astype(bfnp)

    svB = np.zeros((M, 520), f)
    for h in range(HEADS):
        svB[:, 65 * h:65 * h + DH] = smv[h]
        svB[:, 65 * h + DH] = 1.0
    s["svmemB"] = svB.astype(bfnp)

    bkv_v = np.asarray(bkv, f)[DIM:]
    s["b_sv"] = np.broadcast_to(bkv_v[None, :], (128, DIM)).astype(bfnp).copy()
    s["bq"] = np.asarray(bq, f).reshape(DIM, 1).copy()
    s["bk"] = np.asarray(bkv, f)[:DIM].reshape(DIM, 1).copy()
    s["bfcol"] = bfv.reshape(NF, 1).astype(bfnp)
    s["borow"] = np.asarray(bo, f).reshape(1, DIM).astype(bfnp)
    return s


def kernel(inp, x, y, Wq, bq, Wkv, bkv, Wf, bf, Wo, bo, m_k, m_v, Sm_k, Sm_v,
           _trace=False):
    f = np.float32
    nc = _get_nc()
    shared = _prep_shared(Wq, bq, Wkv, bkv, Wf, bf, Wo, bo, m_k, m_v, Sm_k, Sm_v)
    inp = np.asarray(inp, f)
    x = np.asarray(x, f)
    y = np.asarray(y, f)
    in_maps = []
    for b in range(B):
        m = dict(shared)
        m["inpT"] = np.ascontiguousarray(inp[b].T).astype(bfnp)
        m["ctxT"] = np.ascontiguousarray(
            np.concatenate([x[b], y[b]], 0).T).astype(bfnp)
        in_maps.append(m)
    res = bass_utils.run_bass_kernel_spmd(
        nc, in_maps, core_ids=list(range(B)),
        **({"trace": True, "trace_cores": [0]} if _trace else {}))
    out = np.stack([np.asarray(res.results[b]["out"]) for b in range(B)], 0)
    if _trace:
        _CACHE["last_results"] = res
    return out
